# revision 15
# baseline (speedup 1.0000x reference)
"""GNN message-passing kernel for Trainium2, 8 NeuronCores.

Sharding: destination-node partition. Host bin-packs the 50000 nodes into
8 cores x 49 windows of <=128 nodes each, balancing edge counts so every
window's edge list fits a fixed number (TPW) of 128-edge tiles. Node ids are
renamed (pi) so each core's slice is contiguous; the final output is
un-permuted on the host.

Per layer on each core:
  - gather h[src] rows from the replicated table (indirect DMA),
    e = edge_attr' @ [We0;We1;be] on PE (K=3 homogeneous), msg = relu(h+e),
    segment-sum via indicator matmul into a PSUM window of 128 dst nodes
  - z = (1+eps)*h_own + agg; transpose to feature-major; MLP1 on PE
  - BatchNorm batch stats via per-core partial sums + AllReduce (b1 cancels
    inside BN so it is skipped entirely)
  - BN-apply+ReLU fused on ScalarE; MLP2 on PE; next-layer table via AllGather
"""
import sys
import numpy as np

sys.path.insert(0, "/opt/trn_rl_repo")

import concourse.bass as bass
import concourse.bacc as bacc
import concourse.mybir as mybir
import concourse.tile as tile
from concourse import bass_utils
from concourse.masks import make_identity

F32 = mybir.dt.float32
I32 = mybir.dt.int32
OP = mybir.AluOpType
AF = mybir.ActivationFunctionType

N_NODES = 50000
N_EDGES = 300000
EMB = 256
HID = 512
L = 5
NCORES = 8
WPC = 49                      # windows per core
BINS = NCORES * WPC           # 392
ROWS_PC = WPC * 128           # 6272 padded rows per core
BN_EPS = 1e-5

_cache = {}


def _build(tpw: int):
    """Build + compile the SPMD program for a fixed tiles-per-window."""
    if tpw in _cache:
        return _cache[tpw]
    nc = bacc.Bacc("TRN2", target_bir_lowering=False, debug=False,
                   num_devices=NCORES)
    d = {}
    d["type_emb"] = nc.dram_tensor("type_emb", [100, EMB], F32, kind="ExternalInput").ap()
    d["depth_emb"] = nc.dram_tensor("depth_emb", [21, EMB], F32, kind="ExternalInput").ap()
    d["emb_idx"] = nc.dram_tensor("emb_idx", [WPC, 128, 2], I32, kind="ExternalInput").ap()
    d["maskpw"] = nc.dram_tensor("maskpw", [128, WPC], F32, kind="ExternalInput").ap()
    d["esrc"] = nc.dram_tensor("esrc", [WPC, 128, tpw], I32, kind="ExternalInput").ap()
    d["edstl"] = nc.dram_tensor("edstl", [WPC, 128, tpw], F32, kind="ExternalInput").ap()
    d["eattr"] = nc.dram_tensor("eattr", [WPC, 3, tpw * 128], F32, kind="ExternalInput").ap()
    d["we_aug"] = nc.dram_tensor("we_aug", [L, 3, EMB], F32, kind="ExternalInput").ap()
    d["w1"] = nc.dram_tensor("w1", [L, EMB, HID], F32, kind="ExternalInput").ap()
    d["w2"] = nc.dram_tensor("w2", [L, HID, EMB], F32, kind="ExternalInput").ap()
    d["gamma"] = nc.dram_tensor("gamma", [L, HID, 1], F32, kind="ExternalInput").ap()
    d["beta"] = nc.dram_tensor("beta", [L, HID, 1], F32, kind="ExternalInput").ap()
    d["b2"] = nc.dram_tensor("b2", [L, EMB], F32, kind="ExternalInput").ap()
    d["eps"] = nc.dram_tensor("eps", [L, 1], F32, kind="ExternalInput").ap()
    out_ext = nc.dram_tensor("out", [ROWS_PC, EMB], F32, kind="ExternalOutput").ap()

    # chunks of windows for the MLP stage (4 windows = 512 nodes, last = 1)
    chunks = [list(range(c * 4, min(c * 4 + 4, WPC))) for c in range((WPC + 3) // 4)]

    with tile.TileContext(nc) as tc:
        with tc.tile_pool(name="const", bufs=1) as cpool, \
             tc.tile_pool(name="meta", bufs=4) as mpool, \
             tc.tile_pool(name="work", bufs=4) as wpool, \
             tc.tile_pool(name="resid", bufs=1) as rpool, \
             tc.tile_pool(name="stat", bufs=1) as spool, \
             tc.tile_pool(name="psA", bufs=2, space="PSUM") as psA, \
             tc.tile_pool(name="psE", bufs=2, space="PSUM") as psE, \
             tc.tile_pool(name="psT", bufs=1, space="PSUM") as psT, \
             tc.tile_pool(name="psY", bufs=2, space="PSUM") as psY, \
             tc.tile_pool(name="psH", bufs=1, space="PSUM") as psH, \
             tc.tile_pool(name="dram", bufs=1, space="DRAM") as dpool:

            # ---- constants ----
            ident = cpool.tile([128, 128], F32, name="ident")
            make_identity(nc, ident[:])
            iota_i = cpool.tile([128, 128], I32, name="iota_i")
            nc.gpsimd.iota(iota_i[:], pattern=[[1, 128]], base=0, channel_multiplier=0)
            iota_f = cpool.tile([128, 128], F32, name="iota_f")
            nc.vector.tensor_copy(iota_f[:], iota_i[:])
            maskw = cpool.tile([128, WPC], F32, name="maskw")
            nc.sync.dma_start(out=maskw[:], in_=d["maskpw"][:])

            # ---- DRAM scratch ----
            own_tab = dpool.tile([ROWS_PC, EMB], F32, name="own_tab")
            cc_in = dpool.tile([ROWS_PC, EMB], F32, name="cc_in")
            tabs = [dpool.tile([NCORES * ROWS_PC, EMB], F32, name=f"tab{l}",
                               addr_space="Shared") for l in range(L)]
            stats_in = dpool.tile([HID, 2], F32, name="stats_in")
            stats_outs = [dpool.tile([HID, 2], F32, name=f"stats_out{l}",
                                     addr_space="Shared") for l in range(L)]

            # ---- residents ----
            y1t = [rpool.tile([128, ROWS_PC], F32, name=f"y1t{m}") for m in range(4)]
            zT = [rpool.tile([128, 512], F32, name=f"zT{k}", bufs=2) for k in range(2)]

            def emb_phase():
                for t in range(WPC):
                    eidx = mpool.tile([128, 2], I32, name="eidx", tag="eidx")
                    nc.sync.dma_start(out=eidx[:], in_=d["emb_idx"][t])
                    g1 = wpool.tile([128, EMB], F32, name="g1", tag="g1")
                    nc.gpsimd.indirect_dma_start(
                        out=g1[:], out_offset=None, in_=d["type_emb"][:],
                        in_offset=bass.IndirectOffsetOnAxis(ap=eidx[:, 0:1], axis=0))
                    g2 = wpool.tile([128, EMB], F32, name="g2", tag="g2")
                    nc.gpsimd.indirect_dma_start(
                        out=g2[:], out_offset=None, in_=d["depth_emb"][:],
                        in_offset=bass.IndirectOffsetOnAxis(ap=eidx[:, 1:2], axis=0))
                    h0 = wpool.tile([128, EMB], F32, name="h0", tag="h0")
                    nc.vector.tensor_add(out=h0[:], in0=g1[:], in1=g2[:])
                    nc.vector.tensor_scalar_mul(h0[:], h0[:], maskw[:, t:t + 1])
                    nc.sync.dma_start(out=own_tab[t * 128:(t + 1) * 128, :], in_=h0[:])
                    nc.sync.dma_start(out=cc_in[t * 128:(t + 1) * 128, :], in_=h0[:])
                nc.gpsimd.collective_compute(
                    "AllGather", OP.bypass,
                    replica_groups=[list(range(NCORES))],
                    ins=[cc_in[:]], outs=[tabs[0][:]])

            def layer(l):
                tab = tabs[l]  # gather source for this layer (l in 0..L-1 -> tabs[l] holds h_l)
                # --- per-layer constants ---
                wel = cpool.tile([3, EMB], F32, name="wel", tag="wel")
                nc.sync.dma_start(out=wel[:], in_=d["we_aug"][l])
                w1k = []
                for k in range(2):
                    t_ = cpool.tile([128, HID], F32, name=f"w1k{k}", tag=f"w1k{k}")
                    nc.sync.dma_start(out=t_[:], in_=d["w1"][l, k * 128:(k + 1) * 128, :])
                    w1k.append(t_)
                w2k = []
                for k in range(4):
                    t_ = cpool.tile([128, EMB], F32, name=f"w2k{k}", tag=f"w2k{k}")
                    nc.sync.dma_start(out=t_[:], in_=d["w2"][l, k * 128:(k + 1) * 128, :])
                    w2k.append(t_)
                gam, bet = [], []
                for m in range(4):
                    g_ = cpool.tile([128, 1], F32, name=f"gam{m}", tag=f"gam{m}")
                    nc.sync.dma_start(out=g_[:], in_=d["gamma"][l, m * 128:(m + 1) * 128, :])
                    gam.append(g_)
                    b_ = cpool.tile([128, 1], F32, name=f"bet{m}", tag=f"bet{m}")
                    nc.sync.dma_start(out=b_[:], in_=d["beta"][l, m * 128:(m + 1) * 128, :])
                    bet.append(b_)
                lidx = cpool.tile([128, 1], I32, name="lidx", tag="lidx")
                nc.gpsimd.memset(lidx[:], l)
                epsb = cpool.tile([128, 1], F32, name="epsb", tag="epsb")
                nc.gpsimd.indirect_dma_start(
                    out=epsb[:], out_offset=None, in_=d["eps"][:],
                    in_offset=bass.IndirectOffsetOnAxis(ap=lidx[:, 0:1], axis=0))
                nc.vector.tensor_scalar_add(epsb[:], epsb[:], 1.0)
                b2b = cpool.tile([128, EMB], F32, name="b2b", tag="b2b")
                nc.gpsimd.indirect_dma_start(
                    out=b2b[:], out_offset=None, in_=d["b2"][:],
                    in_offset=bass.IndirectOffsetOnAxis(ap=lidx[:, 0:1], axis=0))
                # stats accumulators
                sacc, qacc = [], []
                for m in range(4):
                    s_ = spool.tile([128, 1], F32, name=f"sacc{m}", tag=f"sacc{m}")
                    nc.gpsimd.memset(s_[:], 0.0)
                    sacc.append(s_)
                    q_ = spool.tile([128, 1], F32, name=f"qacc{m}", tag=f"qacc{m}")
                    nc.gpsimd.memset(q_[:], 0.0)
                    qacc.append(q_)

                # ---- pass A: edges + MLP1 + stats ----
                for ch in chunks:
                    for wi, w in enumerate(ch):
                        esrc = mpool.tile([128, tpw], I32, name="esrc_t", tag="esrc_t")
                        nc.sync.dma_start(out=esrc[:], in_=d["esrc"][w])
                        edl = mpool.tile([128, tpw], F32, name="edl_t", tag="edl_t")
                        nc.sync.dma_start(out=edl[:], in_=d["edstl"][w])
                        eat = mpool.tile([3, tpw * 128], F32, name="eat_t", tag="eat_t")
                        nc.sync.dma_start(out=eat[:], in_=d["eattr"][w])
                        hown = wpool.tile([128, EMB], F32, name="hown", tag="hown")
                        nc.sync.dma_start(out=hown[:],
                                          in_=own_tab[w * 128:(w + 1) * 128, :])
                        agg = psA.tile([128, EMB], F32, name="agg", tag="agg")
                        for ti in range(tpw):
                            g = wpool.tile([128, EMB], F32, name="g", tag="g", bufs=8)
                            nc.gpsimd.indirect_dma_start(
                                out=g[:], out_offset=None, in_=tab[:],
                                in_offset=bass.IndirectOffsetOnAxis(
                                    ap=esrc[:, ti:ti + 1], axis=0))
                            ep = psE.tile([128, EMB], F32, name="ep", tag="ep")
                            nc.tensor.matmul(out=ep[:],
                                             lhsT=eat[:, ti * 128:(ti + 1) * 128],
                                             rhs=wel[:], start=True, stop=True)
                            sm = wpool.tile([128, EMB], F32, name="sm", tag="sm")
                            nc.vector.tensor_add(out=sm[:], in0=g[:], in1=ep[:])
                            msg = wpool.tile([128, EMB], F32, name="msg", tag="msg")
                            nc.scalar.activation(msg[:], sm[:], AF.Relu)
                            ind = wpool.tile([128, 128], F32, name="ind", tag="ind")
                            nc.vector.tensor_scalar(
                                out=ind[:], in0=iota_f[:], scalar1=edl[:, ti:ti + 1],
                                scalar2=None, op0=OP.is_equal)
                            nc.tensor.matmul(out=agg[:], lhsT=ind[:], rhs=msg[:],
                                             start=(ti == 0), stop=(ti == tpw - 1))
                        z = wpool.tile([128, EMB], F32, name="z", tag="z")
                        nc.vector.scalar_tensor_tensor(
                            out=z[:], in0=hown[:], scalar=epsb[:, 0:1], in1=agg[:],
                            op0=OP.mult, op1=OP.add)
                        for k in range(2):
                            tp = psT.tile([128, 128], F32, name="tp", tag="tp")
                            nc.tensor.transpose(out=tp[:], in_=z[:, k * 128:(k + 1) * 128],
                                                identity=ident[:])
                            nc.vector.tensor_copy(out=zT[k][:, wi * 128:(wi + 1) * 128],
                                                  in_=tp[:])
                    cs = len(ch) * 128
                    co = ch[0] * 128
                    for m in range(4):
                        py = psY.tile([128, 512], F32, name="py", tag="py")
                        for k in range(2):
                            nc.tensor.matmul(
                                out=py[:, :cs],
                                lhsT=w1k[k][:, m * 128:(m + 1) * 128],
                                rhs=zT[k][:, :cs], start=(k == 0), stop=(k == 1))
                        ys = y1t[m][:, co:co + cs]
                        nc.vector.tensor_copy(out=ys, in_=py[:, :cs])
                        t1 = wpool.tile([128, 1], F32, name="t1", tag="t1")
                        nc.vector.tensor_reduce(t1[:], ys,
                                                axis=mybir.AxisListType.X, op=OP.add)
                        nc.vector.tensor_add(out=sacc[m][:], in0=sacc[m][:], in1=t1[:])
                        scr = wpool.tile([128, 512], F32, name="scr", tag="scr", bufs=2)
                        nc.vector.tensor_tensor(out=scr[:, :cs], in0=ys,
                                                in1=ys, op=OP.mult)
                        t2 = wpool.tile([128, 1], F32, name="t2", tag="t2")
                        nc.vector.tensor_reduce(t2[:], scr[:, :cs],
                                                axis=mybir.AxisListType.X, op=OP.add)
                        nc.vector.tensor_add(out=qacc[m][:], in0=qacc[m][:], in1=t2[:])

                # ---- stats AllReduce + scale/bias ----
                for m in range(4):
                    st = wpool.tile([128, 2], F32, name="st", tag="st")
                    nc.vector.tensor_copy(out=st[:, 0:1], in_=sacc[m][:])
                    nc.vector.tensor_copy(out=st[:, 1:2], in_=qacc[m][:])
                    nc.sync.dma_start(out=stats_in[m * 128:(m + 1) * 128, :], in_=st[:])
                stats_out = stats_outs[l]
                nc.gpsimd.collective_compute(
                    "AllReduce", OP.add, replica_groups=[list(range(NCORES))],
                    ins=[stats_in[:]], outs=[stats_out[:]])
                sca, tbi = [], []
                for m in range(4):
                    st2 = wpool.tile([128, 2], F32, name="st2", tag="st2")
                    nc.sync.dma_start(out=st2[:], in_=stats_out[m * 128:(m + 1) * 128, :])
                    mu = wpool.tile([128, 1], F32, name="mu", tag="mu")
                    nc.vector.tensor_scalar_mul(mu[:], st2[:, 0:1], 1.0 / N_NODES)
                    var = wpool.tile([128, 1], F32, name="var", tag="var")
                    # var = q/N - mu^2 + BN_EPS
                    nc.vector.tensor_scalar_mul(var[:], st2[:, 1:2], 1.0 / N_NODES)
                    msq = wpool.tile([128, 1], F32, name="msq", tag="msq")
                    nc.vector.tensor_tensor(out=msq[:], in0=mu[:], in1=mu[:], op=OP.mult)
                    nc.vector.tensor_tensor(out=var[:], in0=var[:], in1=msq[:], op=OP.subtract)
                    nc.vector.tensor_scalar_add(var[:], var[:], BN_EPS)
                    sd = wpool.tile([128, 1], F32, name="sd", tag="sd")
                    nc.scalar.activation(sd[:], var[:], AF.Sqrt)
                    istd = wpool.tile([128, 1], F32, name="istd", tag="istd")
                    nc.vector.reciprocal(istd[:], sd[:])
                    s_ = spool.tile([128, 1], F32, name=f"sc{m}", tag=f"sc{m}")
                    nc.vector.tensor_tensor(out=s_[:], in0=gam[m][:], in1=istd[:], op=OP.mult)
                    sca.append(s_)
                    tmp = wpool.tile([128, 1], F32, name="tmp", tag="tmp")
                    nc.vector.tensor_tensor(out=tmp[:], in0=mu[:], in1=s_[:], op=OP.mult)
                    tb_ = spool.tile([128, 1], F32, name=f"tb{m}", tag=f"tb{m}")
                    nc.vector.tensor_tensor(out=tb_[:], in0=bet[m][:], in1=tmp[:], op=OP.subtract)
                    tbi.append(tb_)

                # ---- pass B: BN-apply + MLP2 + table write ----
                for ch in chunks:
                    cs = len(ch) * 128
                    co = ch[0] * 128
                    for m in range(4):
                        nc.scalar.activation(y1t[m][:, co:co + cs], y1t[m][:, co:co + cs],
                                             AF.Relu, bias=tbi[m][:, 0:1],
                                             scale=sca[m][:, 0:1])
                    for w in ch:
                        ph = psH.tile([128, EMB], F32, name="ph", tag="ph")
                        for k in range(4):
                            nc.tensor.matmul(out=ph[:],
                                             lhsT=y1t[k][:, w * 128:(w + 1) * 128],
                                             rhs=w2k[k][:], start=(k == 0), stop=(k == 3))
                        hn = wpool.tile([128, EMB], F32, name="hn", tag="hn")
                        nc.vector.tensor_add(out=hn[:], in0=b2b[:], in1=ph[:])
                        if l < L - 1:
                            nc.scalar.activation(hn[:], hn[:], AF.Relu)
                            nc.vector.tensor_scalar_mul(hn[:], hn[:], maskw[:, w:w + 1])
                            nc.sync.dma_start(out=own_tab[w * 128:(w + 1) * 128, :], in_=hn[:])
                            nc.sync.dma_start(out=cc_in[w * 128:(w + 1) * 128, :], in_=hn[:])
                        else:
                            nc.sync.dma_start(out=out_ext[w * 128:(w + 1) * 128, :], in_=hn[:])
                if l < L - 1:
                    nc.gpsimd.collective_compute(
                        "AllGather", OP.bypass,
                        replica_groups=[list(range(NCORES))],
                        ins=[cc_in[:]], outs=[tabs[l + 1][:]])

            emb_phase()
            for l in range(L):
                layer(l)

    nc.compile()
    _cache[tpw] = nc
    return nc


def _host_prep(node_ids, node_depth, edge_index, edge_attr):
    """Bin-pack nodes into (core, window, slot); build per-core arrays."""
    ids = np.asarray(node_ids).astype(np.int64).ravel()
    dep = np.clip(np.asarray(node_depth).astype(np.int64).ravel(), 0, 20)
    src = np.asarray(edge_index[0]).astype(np.int64).ravel()
    dst = np.asarray(edge_index[1]).astype(np.int64).ravel()
    attr = np.asarray(edge_attr, dtype=np.float32)

    deg = np.bincount(dst, minlength=N_NODES)
    order = np.argsort(-deg, kind="stable")
    # snake-deal nodes (sorted by degree desc) across BINS bins
    bin_of = np.empty(N_NODES, np.int32)
    slot_of = np.empty(N_NODES, np.int32)
    counts = np.zeros(BINS, np.int32)
    loads = np.zeros(BINS, np.int64)
    fwd = np.arange(BINS)
    rev = fwd[::-1]
    pos = 0
    rnd = 0
    while pos < N_NODES:
        seq = fwd if rnd % 2 == 0 else rev
        for b in seq:
            if pos >= N_NODES:
                break
            if counts[b] >= 128:
                continue
            v = order[pos]
            bin_of[v] = b
            slot_of[v] = counts[b]
            counts[b] += 1
            loads[b] += deg[v]
            pos += 1
        rnd += 1
    tpw = max(1, int(np.ceil(loads.max() / 128.0)))
    row_of = bin_of.astype(np.int64) * 128 + slot_of  # global padded row id

    esrc = np.zeros((NCORES, WPC, 128, tpw), np.int32)
    edstl = np.full((NCORES, WPC, 128, tpw), -1.0, np.float32)
    eattr = np.zeros((NCORES, WPC, 3, tpw * 128), np.float32)
    ebin = bin_of[dst]
    eorder = np.argsort(ebin, kind="stable")
    bounds = np.searchsorted(ebin[eorder], np.arange(BINS + 1))
    srcrow = row_of[src]
    dslot = slot_of[dst]
    for b in range(BINS):
        c, w = divmod(b, WPC)
        el = eorder[bounds[b]:bounds[b + 1]]
        n = len(el)
        k, p = np.divmod(np.arange(n), 128)
        esrc[c, w, p, k] = srcrow[el]
        edstl[c, w, p, k] = dslot[el].astype(np.float32)
        col = k * 128 + p
        eattr[c, w, 0, col] = attr[el, 0]
        eattr[c, w, 1, col] = attr[el, 1]
        eattr[c, w, 2, col] = 1.0

    emb_idx = np.zeros((NCORES, WPC, 128, 2), np.int32)
    maskpw = np.zeros((NCORES, 128, WPC), np.float32)
    nodes_rows = np.arange(N_NODES)
    c_all, rem = np.divmod(bin_of, WPC)
    emb_idx[c_all, rem, slot_of, 0] = ids[nodes_rows]
    emb_idx[c_all, rem, slot_of, 1] = dep[nodes_rows]
    maskpw[c_all, slot_of, rem] = 1.0
    return tpw, row_of, dict(esrc=esrc, edstl=edstl, eattr=eattr,
                             emb_idx=emb_idx, maskpw=maskpw)


def _prepare(node_ids, node_depth, edge_index, edge_attr, node_type_emb,
             depth_emb, We, be, W1, b1, gamma, beta, W2, b2, eps_param):
    tpw, row_of, per = _host_prep(node_ids, node_depth, edge_index, edge_attr)
    nc = _build(tpw)

    we_aug = np.concatenate([np.asarray(We, np.float32),
                             np.asarray(be, np.float32)[:, None, :]], axis=1)
    common = {
        "type_emb": np.asarray(node_type_emb, np.float32),
        "depth_emb": np.asarray(depth_emb, np.float32),
        "we_aug": we_aug,
        "w1": np.asarray(W1, np.float32),
        "w2": np.asarray(W2, np.float32),
        "gamma": np.asarray(gamma, np.float32).reshape(L, HID, 1),
        "beta": np.asarray(beta, np.float32).reshape(L, HID, 1),
        "b2": np.asarray(b2, np.float32),
        "eps": np.asarray(eps_param, np.float32).reshape(L, 1),
    }
    in_maps = []
    for c in range(NCORES):
        m = dict(common)
        m["emb_idx"] = per["emb_idx"][c]
        m["maskpw"] = per["maskpw"][c]
        m["esrc"] = per["esrc"][c]
        m["edstl"] = per["edstl"][c]
        m["eattr"] = per["eattr"][c]
        in_maps.append(m)
    return nc, in_maps, row_of


def _assemble(res, row_of):
    full = np.concatenate([res.results[c]["out"] for c in range(NCORES)], axis=0)
    return full[row_of].astype(np.float32)


def kernel(**inputs):
    nc, in_maps, row_of = _prepare(**inputs)
    res = bass_utils.run_bass_kernel_spmd(nc, in_maps, core_ids=list(range(NCORES)))
    return _assemble(res, row_of)


# revision 22
# speedup vs baseline: 1.4158x; 1.4158x over previous
"""GNN message-passing kernel for Trainium2, 8 NeuronCores.

Sharding: destination-node partition. Host bin-packs the 50000 nodes into
8 cores x 49 windows of <=128 nodes each, balancing edge counts so every
window's edge list fits a fixed number (TPW) of 128-edge tiles. Node ids are
renamed (pi) so each core's slice is contiguous; the final output is
un-permuted on the host.

Per layer on each core:
  - gather h[src] rows from the replicated table (indirect DMA),
    e = edge_attr' @ [We0;We1;be] on PE (K=3 homogeneous), msg = relu(h+e),
    segment-sum via indicator matmul into a PSUM window of 128 dst nodes
  - z = (1+eps)*h_own + agg; transpose to feature-major; MLP1 on PE
  - BatchNorm batch stats via per-core partial sums + AllReduce (b1 cancels
    inside BN so it is skipped entirely)
  - BN-apply+ReLU fused on ScalarE; MLP2 on PE; next-layer table via AllGather
"""
import sys
import numpy as np

sys.path.insert(0, "/opt/trn_rl_repo")

import concourse.bass as bass
import concourse.bacc as bacc
import concourse.mybir as mybir
import concourse.tile as tile
from concourse import bass_utils
from concourse.masks import make_identity

F32 = mybir.dt.float32
F32R = mybir.dt.float32r
I32 = mybir.dt.int32


def R(ap):
    """Reinterpret an fp32 AP as float32r for full-rate PE matmuls."""
    return ap.bitcast(F32R)
OP = mybir.AluOpType
AF = mybir.ActivationFunctionType

N_NODES = 50000
N_EDGES = 300000
EMB = 256
HID = 512
L = 5
NCORES = 8
WPC = 49                      # windows per core
BINS = NCORES * WPC           # 392
ROWS_PC = WPC * 128           # 6272 padded rows per core
BN_EPS = 1e-5

_cache = {}


def _build(tpw: int):
    """Build + compile the SPMD program for a fixed tiles-per-window."""
    if tpw in _cache:
        return _cache[tpw]
    nc = bacc.Bacc("TRN2", target_bir_lowering=False, debug=False,
                   num_devices=NCORES)
    d = {}
    d["type_emb"] = nc.dram_tensor("type_emb", [100, EMB], F32, kind="ExternalInput").ap()
    d["depth_emb"] = nc.dram_tensor("depth_emb", [21, EMB], F32, kind="ExternalInput").ap()
    d["emb_idx"] = nc.dram_tensor("emb_idx", [WPC, 128, 2], I32, kind="ExternalInput").ap()
    d["maskpw"] = nc.dram_tensor("maskpw", [128, WPC], F32, kind="ExternalInput").ap()
    d["esrc"] = nc.dram_tensor("esrc", [WPC, 128, tpw], I32, kind="ExternalInput").ap()
    d["edstl"] = nc.dram_tensor("edstl", [WPC, 128, tpw], F32, kind="ExternalInput").ap()
    d["eattr"] = nc.dram_tensor("eattr", [WPC, 3, tpw * 128], F32, kind="ExternalInput").ap()
    d["we_aug"] = nc.dram_tensor("we_aug", [L, 3, EMB], F32, kind="ExternalInput").ap()
    d["w1"] = nc.dram_tensor("w1", [L, EMB, HID], F32, kind="ExternalInput").ap()
    d["w2"] = nc.dram_tensor("w2", [L, HID, EMB], F32, kind="ExternalInput").ap()
    d["gamma"] = nc.dram_tensor("gamma", [L, HID, 1], F32, kind="ExternalInput").ap()
    d["beta"] = nc.dram_tensor("beta", [L, HID, 1], F32, kind="ExternalInput").ap()
    d["b2"] = nc.dram_tensor("b2", [L, EMB], F32, kind="ExternalInput").ap()
    d["eps"] = nc.dram_tensor("eps", [L, 1], F32, kind="ExternalInput").ap()
    out_ext = nc.dram_tensor("out", [ROWS_PC, EMB], F32, kind="ExternalOutput").ap()

    # chunks of windows for the MLP stage (4 windows = 512 nodes, last = 1)
    chunks = [list(range(c * 4, min(c * 4 + 4, WPC))) for c in range((WPC + 3) // 4)]

    with tile.TileContext(nc) as tc:
        with tc.tile_pool(name="const", bufs=1) as cpool, \
             tc.tile_pool(name="meta", bufs=4) as mpool, \
             tc.tile_pool(name="work", bufs=4) as wpool, \
             tc.tile_pool(name="resid", bufs=1) as rpool, \
             tc.tile_pool(name="stat", bufs=1) as spool, \
             tc.tile_pool(name="psA", bufs=2, space="PSUM") as psA, \
             tc.tile_pool(name="psE", bufs=2, space="PSUM") as psE, \
             tc.tile_pool(name="psT", bufs=1, space="PSUM") as psT, \
             tc.tile_pool(name="psY", bufs=2, space="PSUM") as psY, \
             tc.tile_pool(name="psH", bufs=1, space="PSUM") as psH, \
             tc.tile_pool(name="dram", bufs=1, space="DRAM") as dpool:

            # ---- constants ----
            ident = cpool.tile([128, 128], F32, name="ident")
            make_identity(nc, ident[:])
            identr = cpool.tile([128, 128], F32R, name="identr")
            nc.vector.tensor_copy(out=identr[:], in_=ident[:])
            iota_i = cpool.tile([128, 128], I32, name="iota_i")
            nc.gpsimd.iota(iota_i[:], pattern=[[1, 128]], base=0, channel_multiplier=0)
            iota_f = cpool.tile([128, 128], F32, name="iota_f")
            nc.vector.tensor_copy(iota_f[:], iota_i[:])
            maskw = cpool.tile([128, WPC], F32, name="maskw")
            nc.sync.dma_start(out=maskw[:], in_=d["maskpw"][:])

            # ---- DRAM scratch ----
            own_tab = dpool.tile([ROWS_PC, EMB], F32, name="own_tab")
            cc_in = dpool.tile([ROWS_PC, EMB], F32, name="cc_in")
            tabs = [dpool.tile([NCORES * ROWS_PC, EMB], F32, name=f"tab{l}",
                               addr_space="Shared") for l in range(L)]
            stats_in = dpool.tile([HID, 2], F32, name="stats_in")
            stats_outs = [dpool.tile([HID, 2], F32, name=f"stats_out{l}",
                                     addr_space="Shared") for l in range(L)]

            # ---- residents ----
            y1t = [rpool.tile([128, ROWS_PC], F32R, name=f"y1t{m}") for m in range(4)]
            zT = [rpool.tile([128, 512], F32R, name=f"zT{k}", bufs=2) for k in range(2)]

            def emb_phase():
                for t in range(WPC):
                    eidx = mpool.tile([128, 2], I32, name="eidx", tag="eidx")
                    nc.sync.dma_start(out=eidx[:], in_=d["emb_idx"][t])
                    g1 = wpool.tile([128, EMB], F32, name="g1", tag="g1")
                    nc.gpsimd.indirect_dma_start(
                        out=g1[:], out_offset=None, in_=d["type_emb"][:],
                        in_offset=bass.IndirectOffsetOnAxis(ap=eidx[:, 0:1], axis=0))
                    g2 = wpool.tile([128, EMB], F32, name="g2", tag="g2")
                    nc.gpsimd.indirect_dma_start(
                        out=g2[:], out_offset=None, in_=d["depth_emb"][:],
                        in_offset=bass.IndirectOffsetOnAxis(ap=eidx[:, 1:2], axis=0))
                    h0 = wpool.tile([128, EMB], F32, name="h0", tag="h0")
                    nc.vector.tensor_add(out=h0[:], in0=g1[:], in1=g2[:])
                    nc.vector.tensor_scalar_mul(h0[:], h0[:], maskw[:, t:t + 1])
                    nc.sync.dma_start(out=own_tab[t * 128:(t + 1) * 128, :], in_=h0[:])
                    nc.sync.dma_start(out=cc_in[t * 128:(t + 1) * 128, :], in_=h0[:])
                nc.gpsimd.collective_compute(
                    "AllGather", OP.bypass,
                    replica_groups=[list(range(NCORES))],
                    ins=[cc_in[:]], outs=[tabs[0][:]])

            def layer(l):
                tab = tabs[l]  # gather source for this layer (l in 0..L-1 -> tabs[l] holds h_l)
                # --- per-layer constants ---
                wel = cpool.tile([3, EMB], F32R, name="wel", tag="wel")
                nc.sync.dma_start(out=wel[:], in_=d["we_aug"][l].bitcast(F32R))
                w1k = []
                for k in range(2):
                    t_ = cpool.tile([128, HID], F32R, name=f"w1k{k}", tag=f"w1k{k}")
                    nc.sync.dma_start(out=t_[:], in_=d["w1"][l, k * 128:(k + 1) * 128, :].bitcast(F32R))
                    w1k.append(t_)
                w2k = []
                for k in range(4):
                    t_ = cpool.tile([128, EMB], F32R, name=f"w2k{k}", tag=f"w2k{k}")
                    nc.sync.dma_start(out=t_[:], in_=d["w2"][l, k * 128:(k + 1) * 128, :].bitcast(F32R))
                    w2k.append(t_)
                gam, bet = [], []
                for m in range(4):
                    g_ = cpool.tile([128, 1], F32, name=f"gam{m}", tag=f"gam{m}")
                    nc.sync.dma_start(out=g_[:], in_=d["gamma"][l, m * 128:(m + 1) * 128, :])
                    gam.append(g_)
                    b_ = cpool.tile([128, 1], F32, name=f"bet{m}", tag=f"bet{m}")
                    nc.sync.dma_start(out=b_[:], in_=d["beta"][l, m * 128:(m + 1) * 128, :])
                    bet.append(b_)
                lidx = cpool.tile([128, 1], I32, name="lidx", tag="lidx")
                nc.gpsimd.memset(lidx[:], l)
                epsb = cpool.tile([128, 1], F32, name="epsb", tag="epsb")
                nc.gpsimd.indirect_dma_start(
                    out=epsb[:], out_offset=None, in_=d["eps"][:],
                    in_offset=bass.IndirectOffsetOnAxis(ap=lidx[:, 0:1], axis=0))
                nc.vector.tensor_scalar_add(epsb[:], epsb[:], 1.0)
                b2b = cpool.tile([128, EMB], F32, name="b2b", tag="b2b")
                nc.gpsimd.indirect_dma_start(
                    out=b2b[:], out_offset=None, in_=d["b2"][:],
                    in_offset=bass.IndirectOffsetOnAxis(ap=lidx[:, 0:1], axis=0))
                # stats accumulators
                sacc, qacc = [], []
                for m in range(4):
                    s_ = spool.tile([128, 1], F32, name=f"sacc{m}", tag=f"sacc{m}")
                    nc.gpsimd.memset(s_[:], 0.0)
                    sacc.append(s_)
                    q_ = spool.tile([128, 1], F32, name=f"qacc{m}", tag=f"qacc{m}")
                    nc.gpsimd.memset(q_[:], 0.0)
                    qacc.append(q_)

                # ---- pass A: edges + MLP1 + stats ----
                for ch in chunks:
                    for wi, w in enumerate(ch):
                        esrc = mpool.tile([128, tpw], I32, name="esrc_t", tag="esrc_t")
                        nc.sync.dma_start(out=esrc[:], in_=d["esrc"][w])
                        edl = mpool.tile([128, tpw], F32, name="edl_t", tag="edl_t")
                        nc.sync.dma_start(out=edl[:], in_=d["edstl"][w])
                        eat = mpool.tile([3, tpw * 128], F32R, name="eat_t", tag="eat_t")
                        nc.sync.dma_start(out=eat[:], in_=d["eattr"][w].bitcast(F32R))
                        hown = wpool.tile([128, EMB], F32, name="hown", tag="hown")
                        nc.sync.dma_start(out=hown[:],
                                          in_=own_tab[w * 128:(w + 1) * 128, :])
                        agg = psA.tile([128, EMB], F32, name="agg", tag="agg")
                        for ti in range(tpw):
                            g = wpool.tile([128, EMB], F32R, name="g", tag="g", bufs=8)
                            nc.gpsimd.indirect_dma_start(
                                out=g[:], out_offset=None, in_=tab[:].bitcast(F32R),
                                in_offset=bass.IndirectOffsetOnAxis(
                                    ap=esrc[:, ti:ti + 1], axis=0))
                            ep = psE.tile([128, EMB], F32, name="ep", tag="ep")
                            nc.tensor.matmul(out=ep[:],
                                             lhsT=R(eat[:, ti * 128:(ti + 1) * 128]),
                                             rhs=wel[:], start=True, stop=False)
                            nc.tensor.matmul(out=ep[:], lhsT=identr[:], rhs=g[:],
                                             start=False, stop=True)
                            msg = wpool.tile([128, EMB], F32R, name="msg", tag="msg")
                            nc.scalar.activation(msg[:], ep[:], AF.Relu)
                            ind = wpool.tile([128, 128], F32R, name="ind", tag="ind")
                            nc.vector.tensor_scalar(
                                out=ind[:], in0=iota_f[:], scalar1=edl[:, ti:ti + 1],
                                scalar2=None, op0=OP.is_equal)
                            nc.tensor.matmul(out=agg[:], lhsT=ind[:], rhs=msg[:],
                                             start=(ti == 0), stop=(ti == tpw - 1))
                        z = wpool.tile([128, EMB], F32, name="z", tag="z")
                        nc.vector.scalar_tensor_tensor(
                            out=z[:], in0=hown[:], scalar=epsb[:, 0:1], in1=agg[:],
                            op0=OP.mult, op1=OP.add)
                        for k in range(2):
                            tp = psT.tile([128, 128], F32, name="tp", tag="tp")
                            nc.tensor.transpose(out=tp[:], in_=z[:, k * 128:(k + 1) * 128],
                                                identity=ident[:])
                            nc.vector.tensor_copy(out=zT[k][:, wi * 128:(wi + 1) * 128],
                                                  in_=tp[:])
                    cs = len(ch) * 128
                    co = ch[0] * 128
                    for m in range(4):
                        py = psY.tile([128, 512], F32, name="py", tag="py")
                        for k in range(2):
                            nc.tensor.matmul(
                                out=py[:, :cs],
                                lhsT=w1k[k][:, m * 128:(m + 1) * 128],
                                rhs=zT[k][:, :cs], start=(k == 0), stop=(k == 1))
                        ys = y1t[m][:, co:co + cs]
                        t1 = wpool.tile([128, 1], F32, name="t1", tag="t1")
                        nc.scalar.activation(ys, py[:, :cs], AF.Copy, accum_out=t1[:])
                        nc.vector.tensor_add(out=sacc[m][:], in0=sacc[m][:], in1=t1[:])
                        scr = wpool.tile([128, 512], F32, name="scr", tag="scr", bufs=2)
                        t2 = wpool.tile([128, 1], F32, name="t2", tag="t2")
                        nc.scalar.activation(scr[:, :cs], ys.bitcast(F32), AF.Square, accum_out=t2[:])
                        nc.vector.tensor_add(out=qacc[m][:], in0=qacc[m][:], in1=t2[:])

                # ---- stats AllReduce + scale/bias ----
                for m in range(4):
                    st = wpool.tile([128, 2], F32, name="st", tag="st")
                    nc.vector.tensor_copy(out=st[:, 0:1], in_=sacc[m][:])
                    nc.vector.tensor_copy(out=st[:, 1:2], in_=qacc[m][:])
                    nc.sync.dma_start(out=stats_in[m * 128:(m + 1) * 128, :], in_=st[:])
                stats_out = stats_outs[l]
                nc.gpsimd.collective_compute(
                    "AllReduce", OP.add, replica_groups=[list(range(NCORES))],
                    ins=[stats_in[:]], outs=[stats_out[:]])
                sca, tbi = [], []
                for m in range(4):
                    st2 = wpool.tile([128, 2], F32, name="st2", tag="st2")
                    nc.sync.dma_start(out=st2[:], in_=stats_out[m * 128:(m + 1) * 128, :])
                    mu = wpool.tile([128, 1], F32, name="mu", tag="mu")
                    nc.vector.tensor_scalar_mul(mu[:], st2[:, 0:1], 1.0 / N_NODES)
                    var = wpool.tile([128, 1], F32, name="var", tag="var")
                    # var = q/N - mu^2 + BN_EPS
                    nc.vector.tensor_scalar_mul(var[:], st2[:, 1:2], 1.0 / N_NODES)
                    msq = wpool.tile([128, 1], F32, name="msq", tag="msq")
                    nc.vector.tensor_tensor(out=msq[:], in0=mu[:], in1=mu[:], op=OP.mult)
                    nc.vector.tensor_tensor(out=var[:], in0=var[:], in1=msq[:], op=OP.subtract)
                    nc.vector.tensor_scalar_add(var[:], var[:], BN_EPS)
                    sd = wpool.tile([128, 1], F32, name="sd", tag="sd")
                    nc.scalar.activation(sd[:], var[:], AF.Sqrt)
                    istd = wpool.tile([128, 1], F32, name="istd", tag="istd")
                    nc.vector.reciprocal(istd[:], sd[:])
                    s_ = spool.tile([128, 1], F32, name=f"sc{m}", tag=f"sc{m}")
                    nc.vector.tensor_tensor(out=s_[:], in0=gam[m][:], in1=istd[:], op=OP.mult)
                    sca.append(s_)
                    tmp = wpool.tile([128, 1], F32, name="tmp", tag="tmp")
                    nc.vector.tensor_tensor(out=tmp[:], in0=mu[:], in1=s_[:], op=OP.mult)
                    tb_ = spool.tile([128, 1], F32, name=f"tb{m}", tag=f"tb{m}")
                    nc.vector.tensor_tensor(out=tb_[:], in0=bet[m][:], in1=tmp[:], op=OP.subtract)
                    tbi.append(tb_)

                # ---- pass B: BN-apply + MLP2 + table write ----
                for ch in chunks:
                    cs = len(ch) * 128
                    co = ch[0] * 128
                    for m in range(4):
                        nc.scalar.activation(y1t[m][:, co:co + cs],
                                             y1t[m][:, co:co + cs].bitcast(F32),
                                             AF.Relu, bias=tbi[m][:, 0:1],
                                             scale=sca[m][:, 0:1])
                    for w in ch:
                        ph = psH.tile([128, EMB], F32, name="ph", tag="ph")
                        for k in range(4):
                            nc.tensor.matmul(out=ph[:],
                                             lhsT=y1t[k][:, w * 128:(w + 1) * 128],
                                             rhs=w2k[k][:], start=(k == 0), stop=(k == 3))
                        hn = wpool.tile([128, EMB], F32, name="hn", tag="hn")
                        nc.vector.tensor_add(out=hn[:], in0=b2b[:], in1=ph[:])
                        if l < L - 1:
                            nc.scalar.activation(hn[:], hn[:], AF.Relu)
                            nc.vector.tensor_scalar_mul(hn[:], hn[:], maskw[:, w:w + 1])
                            nc.sync.dma_start(out=own_tab[w * 128:(w + 1) * 128, :], in_=hn[:])
                            nc.sync.dma_start(out=cc_in[w * 128:(w + 1) * 128, :], in_=hn[:])
                        else:
                            nc.sync.dma_start(out=out_ext[w * 128:(w + 1) * 128, :], in_=hn[:])
                if l < L - 1:
                    nc.gpsimd.collective_compute(
                        "AllGather", OP.bypass,
                        replica_groups=[list(range(NCORES))],
                        ins=[cc_in[:]], outs=[tabs[l + 1][:]])

            emb_phase()
            for l in range(L):
                layer(l)

    nc.compile()
    _cache[tpw] = nc
    return nc


def _host_prep(node_ids, node_depth, edge_index, edge_attr):
    """Bin-pack nodes into (core, window, slot); build per-core arrays."""
    ids = np.asarray(node_ids).astype(np.int64).ravel()
    dep = np.clip(np.asarray(node_depth).astype(np.int64).ravel(), 0, 20)
    src = np.asarray(edge_index[0]).astype(np.int64).ravel()
    dst = np.asarray(edge_index[1]).astype(np.int64).ravel()
    attr = np.asarray(edge_attr, dtype=np.float32)

    deg = np.bincount(dst, minlength=N_NODES)
    order = np.argsort(-deg, kind="stable")
    # snake-deal nodes (sorted by degree desc) across BINS bins
    bin_of = np.empty(N_NODES, np.int32)
    slot_of = np.empty(N_NODES, np.int32)
    counts = np.zeros(BINS, np.int32)
    loads = np.zeros(BINS, np.int64)
    fwd = np.arange(BINS)
    rev = fwd[::-1]
    pos = 0
    rnd = 0
    while pos < N_NODES:
        seq = fwd if rnd % 2 == 0 else rev
        for b in seq:
            if pos >= N_NODES:
                break
            if counts[b] >= 128:
                continue
            v = order[pos]
            bin_of[v] = b
            slot_of[v] = counts[b]
            counts[b] += 1
            loads[b] += deg[v]
            pos += 1
        rnd += 1
    # swap refinement: pull the max window load down to the mean tile count
    target = int(np.ceil(loads.sum() / BINS / 128.0)) * 128
    members = [list(np.where(bin_of == b)[0]) for b in range(BINS)]
    for _ in range(2000):
        a = int(loads.argmax())
        if loads[a] <= target:
            break
        b = int(loads.argmin())
        na = max(members[a], key=lambda v: deg[v])
        nb = min(members[b], key=lambda v: deg[v])
        if deg[na] <= deg[nb]:
            break
        members[a].remove(na); members[b].remove(nb)
        members[a].append(nb); members[b].append(na)
        loads[a] += deg[nb] - deg[na]
        loads[b] += deg[na] - deg[nb]
    for b in range(BINS):
        for s, v in enumerate(members[b]):
            bin_of[v] = b
            slot_of[v] = s
    tpw = max(1, int(np.ceil(loads.max() / 128.0)))
    row_of = bin_of.astype(np.int64) * 128 + slot_of  # global padded row id

    esrc = np.zeros((NCORES, WPC, 128, tpw), np.int32)
    edstl = np.full((NCORES, WPC, 128, tpw), -1.0, np.float32)
    eattr = np.zeros((NCORES, WPC, 3, tpw * 128), np.float32)
    ebin = bin_of[dst]
    eorder = np.argsort(ebin, kind="stable")
    bounds = np.searchsorted(ebin[eorder], np.arange(BINS + 1))
    srcrow = row_of[src]
    dslot = slot_of[dst]
    for b in range(BINS):
        c, w = divmod(b, WPC)
        el = eorder[bounds[b]:bounds[b + 1]]
        n = len(el)
        k, p = np.divmod(np.arange(n), 128)
        esrc[c, w, p, k] = srcrow[el]
        edstl[c, w, p, k] = dslot[el].astype(np.float32)
        col = k * 128 + p
        eattr[c, w, 0, col] = attr[el, 0]
        eattr[c, w, 1, col] = attr[el, 1]
        eattr[c, w, 2, col] = 1.0

    emb_idx = np.zeros((NCORES, WPC, 128, 2), np.int32)
    maskpw = np.zeros((NCORES, 128, WPC), np.float32)
    nodes_rows = np.arange(N_NODES)
    c_all, rem = np.divmod(bin_of, WPC)
    emb_idx[c_all, rem, slot_of, 0] = ids[nodes_rows]
    emb_idx[c_all, rem, slot_of, 1] = dep[nodes_rows]
    maskpw[c_all, slot_of, rem] = 1.0
    return tpw, row_of, dict(esrc=esrc, edstl=edstl, eattr=eattr,
                             emb_idx=emb_idx, maskpw=maskpw)


def _prepare(node_ids, node_depth, edge_index, edge_attr, node_type_emb,
             depth_emb, We, be, W1, b1, gamma, beta, W2, b2, eps_param):
    tpw, row_of, per = _host_prep(node_ids, node_depth, edge_index, edge_attr)
    nc = _build(tpw)

    we_aug = np.concatenate([np.asarray(We, np.float32),
                             np.asarray(be, np.float32)[:, None, :]], axis=1)
    common = {
        "type_emb": np.asarray(node_type_emb, np.float32),
        "depth_emb": np.asarray(depth_emb, np.float32),
        "we_aug": we_aug,
        "w1": np.asarray(W1, np.float32),
        "w2": np.asarray(W2, np.float32),
        "gamma": np.asarray(gamma, np.float32).reshape(L, HID, 1),
        "beta": np.asarray(beta, np.float32).reshape(L, HID, 1),
        "b2": np.asarray(b2, np.float32),
        "eps": np.asarray(eps_param, np.float32).reshape(L, 1),
    }
    in_maps = []
    for c in range(NCORES):
        m = dict(common)
        m["emb_idx"] = per["emb_idx"][c]
        m["maskpw"] = per["maskpw"][c]
        m["esrc"] = per["esrc"][c]
        m["edstl"] = per["edstl"][c]
        m["eattr"] = per["eattr"][c]
        in_maps.append(m)
    return nc, in_maps, row_of


def _assemble(res, row_of):
    full = np.concatenate([res.results[c]["out"] for c in range(NCORES)], axis=0)
    return full[row_of].astype(np.float32)


def kernel(**inputs):
    nc, in_maps, row_of = _prepare(**inputs)
    res = bass_utils.run_bass_kernel_spmd(nc, in_maps, core_ids=list(range(NCORES)))
    return _assemble(res, row_of)
